# revision 29
# baseline (speedup 1.0000x reference)
"""Trainium2 Bass kernel for the decoder attention block (2x1024x1024, E=1024,
nhead=16, Tk=2048, F=4096, n_ctx mask over first keys).

Sharding: 8 NeuronCores = 2 batches x 4 query-token ranges (256 rows each);
weights replicated and streamed from HBM; per-core self/cross K+V.
All matmul operands are fp16 (weights pre-cast on the host, activations cast
on chip), halving HBM weight traffic; PSUM accumulates fp32 and the residual
stream stays fp32. fp16 runs at the same PE rate as float32r (1 cycle/row)
but with 4x finer mantissa than bf16, keeping rel err ~1e-3.

Self-contained: builds the Bass/Tile program, shards the full inputs on the
host, runs SPMD on cores 0-7 via run_bass_kernel_spmd, reassembles the output.
"""
import sys
if "/opt/trn_rl_repo" not in sys.path:
    sys.path.insert(0, "/opt/trn_rl_repo")


from contextlib import ExitStack

import numpy as np

import concourse.bass as bass
import concourse.mybir as mybir
import concourse.tile as tile
from concourse import bacc
from concourse.masks import make_identity

f32 = mybir.dt.float32
f32r = mybir.dt.float32r
f16 = mybir.dt.float16
AF = mybir.ActivationFunctionType

P = 128
E = 1024
EC = E // P            # 8 feature chunks
TQ = 256               # query tokens per core
TQT = TQ // P          # 2
TM = 2048              # cross-attention memory tokens
F = 4096
FC = F // P            # 32
H = 16
Dh = 64
EPS = 1e-5
CHUNK = 512            # kv processing chunk (tokens)
CT = CHUNK // P        # 4 tiles per chunk


def _r(ap):
    return ap.bitcast(f32r) if ap.dtype == f32 else ap


def build_nc(n_ctx: int, loop_n: int = 0):
    """Build the single-core SPMD program. n_ctx: self-attn context length."""
    uniform_self = n_ctx == 0
    n_ctx_eff = 1024 if uniform_self else int(n_ctx)
    TC = (n_ctx_eff + P - 1) // P     # context tiles
    TCTX = TC * P
    rem = n_ctx_eff - (TC - 1) * P    # valid rows in last tile (1..128)
    need_mask = (rem != P) and not uniform_self

    nc = bacc.Bacc("TRN2", target_bir_lowering=False, debug=False)

    # ---------------- DRAM parameters ----------------
    xq_d = nc.declare_dram_parameter("xq", [TQ, E], f32, isOutput=False)
    xc_d = nc.declare_dram_parameter("xc", [TCTX, E], f32, isOutput=False)
    memT_d = nc.declare_dram_parameter("memT", [E, TM], f16, isOutput=False)
    w_names = ["s_wqT", "s_wkT", "s_wvT", "s_owT", "c_wqT", "c_wkT", "c_wvT", "c_owT"]
    wd = {n: nc.declare_dram_parameter(n, [E, E], f16, isOutput=False) for n in w_names}
    w1T_d = nc.declare_dram_parameter("w1T", [E, F], f16, isOutput=False)
    w2T_d = nc.declare_dram_parameter("w2T", [F, E], f16, isOutput=False)
    cmask_d = nc.declare_dram_parameter("cmask", [P, TC], f32, isOutput=False)
    out_d = nc.declare_dram_parameter("out", [TQ, E], f32, isOutput=True)

    xq_r = xq_d.rearrange("(c p) e -> p c e", p=P)        # [128, TQT, E]
    xc_r = xc_d.rearrange("(c p) e -> p c e", p=P)        # [128, TC, E]
    memT_r = memT_d.rearrange("(c p) t -> p c t", p=P)    # [128, EC, TM]
    w_r = {n: wd[n].rearrange("(c p) m -> p c m", p=P) for n in w_names}
    w1T_r = w1T_d.rearrange("(c p) m -> p c m", p=P)      # [128, EC, F]
    w2T_r = w2T_d.rearrange("(c p) m -> p c m", p=P)      # [128, FC, E]
    out_r = out_d.rearrange("(c p) e -> p c e", p=P)

    ctx = ExitStack()
    with ctx:
        ctx.enter_context(nc.allow_low_precision(reason="float32r rounding intended"))
        tc = ctx.enter_context(tile.TileContext(nc))

        # ---- kernel-lifetime pools ----
        const = ctx.enter_context(tc.tile_pool(name="const", bufs=1))
        xpool = ctx.enter_context(tc.tile_pool(name="x", bufs=1))
        wpool = ctx.enter_context(tc.tile_pool(name="w", bufs=6))
        spool = ctx.enter_context(tc.tile_pool(name="stats", bufs=6))
        # psum: 5 + 2 + 1 = 8 banks
        ps5 = ctx.enter_context(tc.tile_pool(name="ps5", bufs=5, space="PSUM"))
        psS = ctx.enter_context(tc.tile_pool(name="psS", bufs=3, space="PSUM"))

        # ---- constants ----
        ident = const.tile([P, P], f32)
        make_identity(nc, ident)
        ones65 = const.tile([65, Dh], f16)
        nc.vector.memset(ones65[:], 1.0)
        onesv = const.tile([P, CT, H], f16)
        nc.vector.memset(onesv[:], 1.0)
        onesq = const.tile([P, TQ], f16)
        if uniform_self:
            nc.vector.memset(onesq[:], 1.0)
        eps_t = const.tile([P, 1], f32)
        nc.vector.memset(eps_t[:], EPS)
        cmask_t = const.tile([P, TC], f32)
        if need_mask:
            nc.sync.dma_start(out=cmask_t[:], in_=cmask_d[:])

        if loop_n:
            ctx.enter_context(tc.For_i(0, loop_n, 1))

        # ---- persistent residual state ----
        x_q = xpool.tile([P, TQT, E], f32, tag="xq")
        nc.sync.dma_start(out=x_q[:], in_=xq_r[:])

        # ---------------- helpers ----------------
        def load_wh(dram_view, half, nm):
            """load one 512-col half of a (128, EC, 1024) weight group."""
            t = wpool.tile([P, EC, 512], f16, tag="w", name=nm)
            nc.sync.dma_start(out=t[:], in_=dram_view[:, :, half * 512:(half + 1) * 512])
            return t

        def load_w2(dram_view, nm):
            """pair of halves [a, b] covering 1024 cols."""
            return [load_wh(dram_view, 0, nm + "a"), load_wh(dram_view, 1, nm + "b")]

        def wcol(wpair, c0, n):
            """slice cols [c0, c0+n) from a half-pair view."""
            h, off = divmod(c0, 512)
            assert off + n <= 512
            return wpair[h][:, :, off:off + n]

        def layernorm(x_tiles, n_tiles, out_tile):
            """token-major LN: out = (x - mean) * rsqrt(var + eps)."""
            for t in range(n_tiles):
                st = spool.tile([P, 2, 6], f32, tag="bnst")
                xin = x_tiles[:, t, :].rearrange("p (s e) -> p s e", s=2)
                for s in range(2):
                    nc.vector.bn_stats(out=st[:, s, :], in_=xin[:, s, :])
                mv = spool.tile([P, 2], f32, tag="bnmv")
                nc.vector.bn_aggr(out=mv[:], in_=st[:])
                sd = spool.tile([P, 1], f32, tag="bnsd")
                nc.scalar.activation(sd[:], mv[:, 1:2], AF.Sqrt, bias=eps_t[:])
                rstd = spool.tile([P, 1], f32, tag="bnrs")
                nc.vector.reciprocal(rstd[:], sd[:])
                nm = spool.tile([P, 1], f32, tag="bnnm")
                nc.vector.tensor_mul(nm[:], mv[:, 0:1], rstd[:])
                nc.scalar.mul(nm[:], nm[:], -1.0)
                nc.scalar.activation(out_tile[:, t, :], x_tiles[:, t, :], AF.Identity,
                                     bias=nm[:], scale=rstd[:])

        def transpose_in(nx_tiles, n_tiles, outT):
            """(128, n_tiles, E) token-major -> (128, EC, n_tiles*128) feat-major."""
            for e in range(EC):
                for t0 in range(0, n_tiles, 4):
                    tn = min(4, n_tiles - t0)
                    pst = ps5.tile([P, 512], f32, tag="ps5")
                    for i in range(tn):
                        nc.tensor.transpose(
                            pst[:, i * P:(i + 1) * P],
                            nx_tiles[:, t0 + i, e * P:(e + 1) * P], ident[:])
                    nc.scalar.activation(outT[:, e, t0 * P:(t0 + tn) * P],
                                         pst[:, 0:tn * P], AF.Copy)

        def proj_featmajor(srcT, src_cols, wpair, outT, mtiles=EC, act=None):
            """outT[:, m, :cols] = sum_k w[:, k, mP:(m+1)P].T @ srcT[:, k, :cols]."""
            for m in range(mtiles):
                ps = ps5.tile([P, 512], f32, tag="ps5")
                for k in range(EC):
                    nc.tensor.matmul(ps[:, 0:src_cols],
                                     _r(wcol(wpair, m * P, P)[:, k, :]),
                                     _r(srcT[:, k, 0:src_cols]),
                                     start=(k == 0), stop=(k == EC - 1))
                if act is None:
                    nc.scalar.activation(outT[:, m, 0:src_cols], ps[:, 0:src_cols],
                                         AF.Copy)
                else:
                    nc.scalar.activation(outT[:, m, 0:src_cols], ps[:, 0:src_cols], act)

        def attention(qT, wk, wv, kv_chunks, acc65, rdpool, kTpool, vpool, ppool,
                      masked, uniform):
            """Shared self/cross attention.
            qT: (128, EC, TQ); wk/wv: (128, EC, 1024) weight tiles.
            kv_chunks: list of (kvT_tile, col0, ct) feature-major kv slices.
            acc65: (65, H, TQ); rows 0..63 attn out (feature-major per head),
            row 64 softmax denominator. Normalized in place at the end.
            """
            n_chunks = len(kv_chunks)
            for ci, (kvT, c0, ct) in enumerate(kv_chunks):
                first, last = ci == 0, ci == n_chunks - 1
                # K^T chunk (128, EC, ct*P)
                kTc = kTpool.tile([P, EC, CHUNK], f16, tag="kT")
                for m in range(EC):
                    ps = ps5.tile([P, 512], f32, tag="ps5")
                    for k in range(EC):
                        nc.tensor.matmul(ps[:, 0:ct * P],
                                         _r(wcol(wk, m * P, P)[:, k, :]),
                                         _r(kvT[:, k, c0:c0 + ct * P]),
                                         start=(k == 0), stop=(k == EC - 1))
                    nc.vector.tensor_copy(kTc[:, m, 0:ct * P], ps[:, 0:ct * P])
                # V pack (128, ct, H, 65) with ones column at 64
                vpk = vpool.tile([P, CT, H, 65], f16, tag="v")
                nc.vector.tensor_copy(vpk[:, 0:ct, :, 64], onesv[:, 0:ct, :])
                for mt in range(ct):
                    for half in range(2):
                        ps = ps5.tile([P, 512], f32, tag="ps5")
                        for k in range(EC):
                            nc.tensor.matmul(
                                ps[:],
                                _r(kvT[:, k, c0 + mt * P:c0 + (mt + 1) * P]),
                                _r(wv[half][:, k, :]),
                                start=(k == 0), stop=(k == EC - 1))
                        nc.vector.tensor_copy(
                            vpk[:, mt, half * 8:(half + 1) * 8, 0:64],
                            ps.rearrange("p (h d) -> p h d", d=64))
                # per-head scores/softmax/AV
                for h in range(H):
                    ht, hr = h // 2, (h % 2) * Dh
                    probsT = ppool.tile([P, CT, TQ], f16, tag="probs")
                    for kt0 in range(0, ct, 2):
                        kn = min(2, ct - kt0)
                        if not uniform:
                            ps_s = psS.tile([P, 2, TQ], f32, tag="psS")
                            for j in range(kn):
                                kt = kt0 + j
                                nc.tensor.matmul(ps_s[:, j, :],
                                                 _r(kTc[hr:hr + Dh, ht, kt * P:(kt + 1) * P]),
                                                 _r(qT[hr:hr + Dh, ht, :]),
                                                 start=True, stop=True)
                            nc.scalar.activation(probsT[:, kt0:kt0 + kn, :],
                                                 ps_s[:, 0:kn, :], AF.Exp, scale=0.125)
                            if masked and last and kt0 + kn == ct:
                                nc.vector.tensor_scalar_mul(
                                    probsT[:, ct - 1, :], probsT[:, ct - 1, :],
                                    cmask_t[:, TC - 1:TC])
                        else:
                            for j in range(kn):
                                nc.vector.tensor_copy(probsT[:, kt0 + j, :], onesq[:])
                    ps_av = ps5.tile([65, TQ], f32, tag="ps5")
                    for kt in range(ct):
                        nc.tensor.matmul(ps_av[:],
                                         _r(vpk[:, kt, h, :]),
                                         _r(probsT[:, kt, :]),
                                         start=(kt == 0), stop=(kt == ct - 1))
                    if first:
                        nc.vector.tensor_copy(acc65[:, h, :], ps_av[:])
                    else:
                        nc.vector.tensor_add(acc65[:, h, :], acc65[:, h, :], ps_av[:])
            # normalize into f16 accb (uniform-f16 stationary for out_proj)
            accb = rdpool.tile([Dh, H, TQ], f16, tag="accb")
            for h in range(H):
                rd = rdpool.tile([65, TQ], f16, tag="rd")
                nc.vector.reciprocal(rd[64:65, :], acc65[64:65, h, :])
                ps_b = psS.tile([64, TQ], f32, tag="psS")
                nc.tensor.matmul(ps_b[:], _r(ones65[64:65, :]), _r(rd[64:65, :]),
                                 start=True, stop=True)
                nc.vector.tensor_mul(accb[:, h, :], acc65[0:64, h, :], ps_b[:])
            return accb

        def load_ow_quarter(ow_dram, nq, nm):
            """(E,E) out-proj weight, head-major quarter: (64, H, 256)."""
            ow_hm = ow_dram.rearrange("(h d) m -> d h m", d=Dh)
            owq = wpool.tile([Dh, H, 256], f16, tag="w", name=nm)
            nc.sync.dma_start(out=owq[:], in_=ow_hm[:, :, nq * 256:(nq + 1) * 256])
            return owq

        def out_proj(accb, ow_dram, dest, nm):
            """dest[:, tq, :] += attn @ ow (contract over H heads x 64 feats).
            tq-major so dest[:, 0, :] completes early and the next LN can start."""
            for nq in range(4):
                owq = load_ow_quarter(ow_dram, nq, f"{nm}{nq}")
                for tq in range(TQT):
                    ps = psS.tile([P, TQ], f32, tag="psS")
                    for h in range(H):
                        nc.tensor.matmul(
                            ps[:],
                            _r(accb[:, h, tq * P:(tq + 1) * P]),
                            _r(owq[:, h, :]),
                            start=(h == 0), stop=(h == H - 1))
                    nc.vector.tensor_add(dest[:, tq, nq * 256:(nq + 1) * 256],
                                         dest[:, tq, nq * 256:(nq + 1) * 256], ps[:])

        # =========================================================
        # Phase 1: LN1 + transposes (xc scoped here)
        with tc.tile_pool(name="nxT1", bufs=1) as nxT1:
            nxqT = nxT1.tile([P, EC, TQ], f16, tag="nxqT")
            nxcT = nxT1.tile([P, EC, TCTX], f16, tag="nxcT")
            with tc.tile_pool(name="p1", bufs=1) as p1:
                xc_t = p1.tile([P, TC, E], f32, tag="xc")
                nc.sync.dma_start(out=xc_t[:], in_=xc_r[:])
                nx_q = p1.tile([P, TQT, E], f32, tag="nxq")
                layernorm(x_q, TQT, nx_q)
                transpose_in(nx_q, TQT, nxqT)
                nx_c = p1.tile([P, TC, E], f32, tag="nxc")
                layernorm(xc_t, TC, nx_c)
                transpose_in(nx_c, TC, nxcT)

            # Phase 2: self QKV + attention + out_proj
            with tc.tile_pool(name="qT2", bufs=1) as qTp, \
                 tc.tile_pool(name="kT2", bufs=1) as kTp, \
                 tc.tile_pool(name="v2", bufs=1) as vp, \
                 tc.tile_pool(name="pr2", bufs=2) as pp, \
                 tc.tile_pool(name="at2", bufs=1) as ap_, \
                 tc.tile_pool(name="rd2", bufs=1) as rdp:
                wq = load_w2(w_r["s_wqT"][:], "swq")
                qT = qTp.tile([P, EC, TQ], f16, tag="qT")
                proj_featmajor(nxqT, TQ, wq, qT)
                wk = load_w2(w_r["s_wkT"][:], "swk")
                wv = load_w2(w_r["s_wvT"][:], "swv")
                acc65 = ap_.tile([65, H, TQ], f32r, tag="acc65")
                chunks = []
                c0 = 0
                while c0 < TCTX:
                    ct = min(CT, (TCTX - c0) // P)
                    chunks.append((nxcT, c0, ct))
                    c0 += ct * P
                accb = attention(qT, wk, wv, chunks, acc65, rdp, kTp, vp, pp,
                                 need_mask, uniform_self)
                out_proj(accb, wd["s_owT"], x_q, "sow")

        # Phase 3/4: LN2 + transpose, cross QKV + attention + out_proj
        with tc.tile_pool(name="qT4", bufs=1) as qTp, \
             tc.tile_pool(name="kT4", bufs=1) as kTp, \
             tc.tile_pool(name="v4", bufs=1) as vp, \
             tc.tile_pool(name="pr4", bufs=2) as pp, \
             tc.tile_pool(name="at4", bufs=1) as ap_, \
             tc.tile_pool(name="rd4", bufs=1) as rdp, \
             tc.tile_pool(name="m4", bufs=1) as mp:
            with tc.tile_pool(name="nxT3", bufs=1) as nxT3:
                nx2T = nxT3.tile([P, EC, TQ], f16, tag="nx2T")
                with tc.tile_pool(name="p3", bufs=1) as p3:
                    nx2 = p3.tile([P, TQT, E], f32, tag="nx2")
                    layernorm(x_q, TQT, nx2)
                    transpose_in(nx2, TQT, nx2T)
                cwq = load_w2(w_r["c_wqT"][:], "cwq")
                qT = qTp.tile([P, EC, TQ], f16, tag="qT")
                proj_featmajor(nx2T, TQ, cwq, qT)
            if True:
                wk = load_w2(w_r["c_wkT"][:], "cwk")
                wv = load_w2(w_r["c_wvT"][:], "cwv")
                acc65 = ap_.tile([65, H, TQ], f32r, tag="acc65")
                # memory chunks streamed from DRAM
                chunks = []
                for c0 in range(0, TM, CHUNK):
                    mt_ = mp.tile([P, EC, CHUNK], f16, tag="mem")
                    nc.sync.dma_start(out=mt_[:], in_=memT_r[:, :, c0:c0 + CHUNK])
                    chunks.append((mt_, 0, CT))
                accb = attention(qT, wk, wv, chunks, acc65, rdp, kTp, vp, pp,
                                 False, False)
                out_proj(accb, wd["c_owT"], x_q, "cow")

        # Phase 5: LN3 + transpose; Phase 6: FFN
        with tc.tile_pool(name="nxT5", bufs=1) as nxT5:
            nx3T = nxT5.tile([P, EC, TQ], f16, tag="nx3T")
            with tc.tile_pool(name="p5", bufs=1) as p5:
                nx3 = p5.tile([P, TQT, E], f32, tag="nx3")
                layernorm(x_q, TQT, nx3)
                transpose_in(nx3, TQT, nx3T)

            with tc.tile_pool(name="hT", bufs=1) as hp:
                hT = hp.tile([P, FC, TQ], f16, tag="hT")
                for g in range(8):
                    w1g = wpool.tile([P, EC, 512], f16, tag="w", name=f"w1g{g}")
                    nc.sync.dma_start(out=w1g[:],
                                      in_=w1T_r[:, :, g * 512:(g + 1) * 512])
                    for m in range(4):
                        ps = ps5.tile([P, 512], f32, tag="ps5")
                        for k in range(EC):
                            nc.tensor.matmul(ps[:, 0:TQ],
                                             _r(w1g[:, k, m * P:(m + 1) * P]),
                                             _r(nx3T[:, k, :]),
                                             start=(k == 0), stop=(k == EC - 1))
                        nc.scalar.activation(hT[:, g * 4 + m, :], ps[:, 0:TQ],
                                             AF.Gelu)
                # y = hT.T @ w2T, accumulated over all 32 k-tiles
                ps_y = [[ps5.tile([P, 512], f32, tag="ps5", name=f"psy_{tq}_{nh}")
                         for nh in range(2)] for tq in range(TQT)]
                for g in range(8):
                    w2g = wpool.tile([P, 4, E], f16, tag="w", name=f"w2g{g}")
                    nc.sync.dma_start(out=w2g[:], in_=w2T_r[:, g * 4:(g + 1) * 4, :])
                    for tq in range(TQT):
                        for nh in range(2):
                            for k in range(4):
                                kk = g * 4 + k
                                nc.tensor.matmul(
                                    ps_y[tq][nh][:],
                                    _r(hT[:, kk, tq * P:(tq + 1) * P]),
                                    _r(w2g[:, k, nh * 512:(nh + 1) * 512]),
                                    start=(kk == 0), stop=(kk == FC - 1))
                for tq in range(TQT):
                    for nh in range(2):
                        nc.vector.tensor_add(x_q[:, tq, nh * 512:(nh + 1) * 512],
                                             x_q[:, tq, nh * 512:(nh + 1) * 512],
                                             ps_y[tq][nh][:])

        nc.sync.dma_start(out=out_r[:], in_=x_q[:])

    nc.finalize()
    return nc


# ======================= host side =======================

def host_prep(inputs):
    """Fold LN affine into weights; transpose weights; slice per core.
    Returns (n_ctx, in_maps list of 8 dicts)."""
    tgt = np.asarray(inputs["tgt"], np.float32)
    memory = np.asarray(inputs["memory"], np.float32)
    n_ctx = int(np.asarray(inputs["n_ctx"]))
    ln1_g = np.asarray(inputs["ln1_g"], np.float32)
    ln1_b = np.asarray(inputs["ln1_b"], np.float32)
    ln2_g = np.asarray(inputs["ln2_g"], np.float32)
    ln2_b = np.asarray(inputs["ln2_b"], np.float32)
    ln3_g = np.asarray(inputs["ln3_g"], np.float32)
    ln3_b = np.asarray(inputs["ln3_b"], np.float32)

    def fold(w, b, g, lb):
        # y = ln_raw(x)@ (w*g).T + (w@lb + b)
        w_eff = w * g[None, :]
        b_eff = w @ lb + b
        return w_eff, b_eff

    s_w = np.asarray(inputs["self_w"], np.float32)
    s_b = np.asarray(inputs["self_b"], np.float32)
    c_w = np.asarray(inputs["cross_w"], np.float32)
    c_b = np.asarray(inputs["cross_b"], np.float32)
    s_ow = np.asarray(inputs["self_ow"], np.float32)
    s_ob = np.asarray(inputs["self_ob"], np.float32)
    c_ow = np.asarray(inputs["cross_ow"], np.float32)
    c_ob = np.asarray(inputs["cross_ob"], np.float32)
    w1 = np.asarray(inputs["w1"], np.float32)
    b1 = np.asarray(inputs["b1"], np.float32)
    w2 = np.asarray(inputs["w2"], np.float32)
    b2 = np.asarray(inputs["b2"], np.float32)

    s_wq, s_bq = fold(s_w[:E], s_b[:E], ln1_g, ln1_b)
    s_wk, s_bk = fold(s_w[E:2 * E], s_b[E:2 * E], ln1_g, ln1_b)
    s_wv, s_bv = fold(s_w[2 * E:], s_b[2 * E:], ln1_g, ln1_b)
    c_wq, c_bq = fold(c_w[:E], c_b[:E], ln2_g, ln2_b)
    # cross k/v act on raw memory (no LN)
    c_wk, c_bk = c_w[E:2 * E], c_b[E:2 * E]
    c_wv, c_bv = c_w[2 * E:], c_b[2 * E:]
    w1_eff, b1_eff = fold(w1, b1, ln3_g, ln3_b)

    zero_bias = not (np.any(s_bq) or np.any(s_bk) or np.any(s_bv) or np.any(s_ob)
                     or np.any(c_bq) or np.any(c_bk) or np.any(c_bv) or np.any(c_ob)
                     or np.any(b1_eff) or np.any(b2))
    if not zero_bias:
        raise NotImplementedError("nonzero biases not supported by this kernel")

    uniform = n_ctx == 0
    n_ctx_eff = 1024 if uniform else n_ctx
    TC = (n_ctx_eff + P - 1) // P
    TCTX = TC * P

    def b16(a):
        return np.ascontiguousarray(a.T).astype(np.float16)

    shared = {
        "memT": None,  # per batch
        "s_wqT": b16(s_wq),
        "s_wkT": b16(s_wk),
        "s_wvT": b16(s_wv),
        "s_owT": b16(s_ow),
        "c_wqT": b16(c_wq),
        "c_wkT": b16(c_wk),
        "c_wvT": b16(c_wv),
        "c_owT": b16(c_ow),
        "w1T": b16(w1_eff),
        "w2T": b16(w2),
    }
    # context mask for partial last tile
    cmask = np.ones((P, TC), np.float32)
    nvalid = n_ctx_eff - (TC - 1) * P
    cmask[nvalid:, TC - 1] = 0.0

    memT = [np.ascontiguousarray(memory[g].T).astype(np.float16) for g in range(2)]
    xcs = []
    for g in range(2):
        xc = tgt[g, :min(n_ctx_eff, 1024)]
        if xc.shape[0] < TCTX:
            xc = np.concatenate([xc, np.zeros((TCTX - xc.shape[0], E), np.float32)], 0)
        xcs.append(np.ascontiguousarray(xc))

    in_maps = []
    for c in range(8):
        g, r0 = c // 4, (c % 4) * TQ
        m = dict(shared)
        m["memT"] = memT[g]
        m["xq"] = np.ascontiguousarray(tgt[g, r0:r0 + TQ])
        m["xc"] = xcs[g]
        m["cmask"] = cmask
        in_maps.append(m)
    return n_ctx, in_maps


def assemble(results):
    out = np.empty((2, 1024, E), np.float32)
    for c in range(8):
        g, r0 = c // 4, (c % 4) * TQ
        out[g, r0:r0 + TQ] = results[c]["out"]
    return out


_NC_CACHE = {}


def kernel(**inputs):
    """Full (unsharded) inputs -> full (2, 1024, 1024) float32 output."""
    from concourse.bass_utils import run_bass_kernel_spmd
    n_ctx, in_maps = host_prep(inputs)
    nc = _NC_CACHE.get(n_ctx)
    if nc is None:
        nc = build_nc(n_ctx)
        _NC_CACHE[n_ctx] = nc
    res = run_bass_kernel_spmd(nc, in_maps, list(range(8)))
    return assemble(res.results)

